# revision 44
# baseline (speedup 1.0000x reference)
"""Neural Tensor Network (NTN) scoring kernel for Trainium2 (Bass/Tile).

score_k(e1, e2, r) = u_k . tanh( e1^T W[r,k] e2 + v_k . [e1;e2] + b_k )
pred = sigmoid( sum_k score_k )

Strategy (v2)
-------------
Host: group the batch by relation id, pack each group into 32-item slots
(PE column-strip granularity), and greedily balance the slots across the
8 cores.  All per-relation parameters except u are folded into one
augmented bf16 table XTb[r] of shape [101, 4*102] such that with
e1~ = [e1; 1]:

    P[k*102 + j] = (e1^T W_k)[j] + v_k^b[j]     (j < 100)
    P[k*102+100] = v_k^a . e1 + b_k
    P[k*102+101] = 0                             (alignment pad)

so with e2~ = [e2; 1; 0]:  g_pre_k = sum_j P[k*102+j] * e2~[j]
and  pred = sigmoid( sum_k u_k * tanh(g_pre_k) ).  u stays f32 in a
separate per-lane table (zeros on padding lanes, which also neutralises
garbage rows).

Device (one SPMD program on 8 cores):
  * entity rows are gathered on-device (SWDGE indirect, f32->bf16 cast),
    scattered into padded slot order through a DRAM bounce buffer, and
    read back with a rearranged AP,
  * the core's whole slot-ordered XT shard streams into SBUF in 4-slot
    chunks (3264B per-partition descriptor runs -- kept under 4KB so the
    HWDGE spreads them over all 16 SDMA engines; bigger runs all pin to
    one engine at ~27GB/s, which was the v1 bottleneck),
  * per 128-row block: PE transposes the e1~ rows (bf16), four matmuls
    (one per 32-item slot, packed into the four column strips of one
    PSUM tile) produce P, VectorE does the segmented e2~ multiply+reduce,
  * one batched tanh / u-multiply / k-reduce / sigmoid tail.
"""

import sys
from contextlib import ExitStack

for _p in ("/opt/trn_rl_repo", "/opt/trn_rl_repo/concourse"):
    if _p not in sys.path:
        sys.path.insert(0, _p)

import numpy as np  # noqa: E402
import ml_dtypes  # noqa: E402

import concourse.bass as bass  # noqa: E402
import concourse.mybir as mybir  # noqa: E402
import concourse.tile as tile  # noqa: E402
from concourse.bass import IndirectOffsetOnAxis  # noqa: E402
from concourse.masks import make_identity  # noqa: E402

F32 = mybir.dt.float32
BF16 = mybir.dt.bfloat16
FP8 = mybir.dt.float8e4
I32 = mybir.dt.int32
BF16_NP = ml_dtypes.bfloat16
FP8_NP = ml_dtypes.float8_e4m3

B = 4096
D = 100
K = 4
NREL = 1000
NENT = 100000
NCORES = 8
DA = D + 1           # augmented contraction dim (e1; 1)
DAP = 104            # DA padded to a multiple of 8: DMAs whose per-partition
                     # descriptor count is not a multiple of 8 all land on ONE
                     # SDMA engine (~27GB/s); 104 rows spread over 13 engines
DJ = DA + 1          # 102: padded e2~ segment (e1^T W | bias | 0)
NW = K * DJ          # 408 folded W/V/B columns (bf16)
SLOT = 32            # items per slot (PE col-strip granularity)
CH = 16              # slots per XT fetch chunk (4 blocks)
CAP = 512            # per-core item capacity (hsd layout is [128, 4])
DCOL = CAP // 128
# ep_all row layout (bf16): [e1 (0:100) | 1 | pad3 | e2 (104:204) | 1 | pad3]
RW = 208
E2OFF = 104          # e2~ segment start (4B aligned), 102 cols used


# ---------------------------------------------------------------------------
# Walrus on this toolchain rejects instructions carrying more than one
# sync-wait command. After Tile schedules, move any excess waits onto
# freshly inserted same-engine nops placed directly before the instruction
# (engines execute their stream in order, so semantics are unchanged).
# ---------------------------------------------------------------------------
_WAIT_LIMIT = 1
_split_counter = [0]


def _split_excess_waits(nc):
    for f in nc.m.functions:
        for blk in f.blocks:
            il = blk.instructions
            k = 0
            while k < len(il):
                inst = il[k]
                si = inst.sync_info
                if si is not None and si.on_wait and len(si.on_wait) > _WAIT_LIMIT:
                    waits = list(si.on_wait)
                    excess = waits[:-_WAIT_LIMIT]
                    del si.on_wait[:-_WAIT_LIMIT]
                    for w in excess:
                        _split_counter[0] += 1
                        nop = mybir.InstNoOp(
                            name=f"I-waitsplit-{_split_counter[0]}", ins=[], outs=[])
                        nop.engine = inst.engine
                        nop.sync_info = mybir.SyncInfo(on_wait=[w], on_update=[])
                        nc.register_instruction(nop, overwrite=True)
                        il.insert(k, nop)
                        k += 1
                k += 1


_orig_tile_exit = tile.TileContext.__exit__


def _patched_tile_exit(self, exc_type, exc, tb):
    r = _orig_tile_exit(self, exc_type, exc, tb)
    if exc_type is None:
        _split_excess_waits(self.nc)
    return r


if getattr(tile.TileContext, "_ant_wait_split_patch", False) is False:
    tile.TileContext.__exit__ = _patched_tile_exit
    tile.TileContext._ant_wait_split_patch = True


# ---------------------------------------------------------------------------
# Host-side preparation
# ---------------------------------------------------------------------------
def _build_xt(W, V, Bp):
    """Fold W/V/Bp into the augmented relation table XTb [NREL, DAP, NW] fp8.

    fp8e4m3 keeps ~2 decimal digits; the bilinear scores are ~1e3 with sigma
    ~15 so tanh is saturated far beyond fp8's error, and u (the only factor
    the final sigmoid is sensitive to) stays f32 in a separate table.
    """
    core = np.zeros((NREL, DAP, K, DJ), np.float32)
    core[:, :D, :, :D] = W.transpose(0, 2, 1, 3)          # [r, d, k, e]
    core[:, D, :, :D] = V[:, :, D:]                        # v^b
    core[:, :D, :, D] = V[:, :, :D].transpose(0, 2, 1)     # v^a
    core[:, D, :, D] = Bp
    return core.reshape(NREL, DAP, NW).astype(FP8_NP)


def _route(heads, tails, relations, U):
    """Group by relation, pack 32-item slots, balance slots across cores."""
    order = np.argsort(relations, kind="stable")
    rels = relations[order]
    # global slot list: (relation id, item indices)
    slots = []
    i = 0
    n = len(order)
    while i < n:
        j = i
        while j < n and rels[j] == rels[i]:
            j += 1
        for a in range(i, j, SLOT):
            slots.append((int(rels[i]), order[a:min(a + SLOT, j)]))
        i = j

    # greedy balance: total items exactly fills NCORES*CAP, so items are the
    # binding constraint -- place big slots first into the core with most
    # remaining capacity (ties: fewest slots), which also balances slot counts
    core_slots = [[] for _ in range(NCORES)]
    core_items = [0] * NCORES
    for s in sorted(slots, key=lambda s: -len(s[1])):
        c = min(range(NCORES),
                key=lambda c: (core_items[c] + len(s[1]) > CAP,
                               -(CAP - core_items[c]), len(core_slots[c])))
        if core_items[c] + len(s[1]) > CAP:
            raise RuntimeError("slot does not fit on any core")
        core_slots[c].append(s)
        core_items[c] += len(s[1])

    # split each core's slot list into 4 quarters of EXACTLY 128 items
    # (splitting the straddling slot keeps this always feasible).  Quarter q
    # becomes dense column q, and its blocks get their own bounce tile, so
    # scatter q depends only on column q's gathers and read-back q only on
    # scatter q.
    core_quarters = []
    for cs in core_slots:
        # deal slots into the 4 quarters balancing both items (exactly 128
        # each; slots split across quarters when needed) and slot counts
        quarters = [[] for _ in range(DCOL)]
        items = [0] * DCOL
        for rr, idxs in sorted(cs, key=lambda s: -len(s[1])):
            idxs = list(idxs)
            while idxs:
                q = min(range(DCOL),
                        key=lambda q: (items[q] >= 128, len(quarters[q]),
                                       -(128 - items[q])))
                assert items[q] < 128
                take = min(128 - items[q], len(idxs))
                quarters[q].append((rr, np.asarray(idxs[:take])))
                items[q] += take
                idxs = idxs[take:]
        assert all(n == 128 for n in items)
        core_quarters.append(quarters)

    QB = max((len(q) + 3) // 4
             for quarters in core_quarters for q in quarters)
    NBLK = DCOL * QB
    S = NBLK * 4

    routed = []
    for c in range(NCORES):
        quarters = core_quarters[c]
        slot_rels = np.zeros(S, np.int64)
        have_slot = np.zeros(S, np.bool_)
        # dense-order entity ids + dense -> padded-slot scatter targets.
        # bounce_q is partition-major [128 lanes, QB+1 blocks]: row index
        # p*(QB+1) + local block, read back as 128 contiguous runs
        hsd = np.zeros((128, DCOL), np.int32)
        tsd = np.zeros((128, DCOL), np.int32)
        scat = np.zeros((128, DCOL), np.int32)
        for p in range(128):
            for cc in range(DCOL):
                scat[p, cc] = p * (QB + 1) + QB   # dump col for unused
        ub = np.zeros((128, NBLK * K), np.float32)
        placement = []  # (orig batch index, block, partition row)
        for q, qslots in enumerate(quarters):
            di = 0
            for sq, (rr, idxs) in enumerate(qslots):
                s = q * QB * 4 + sq
                slot_rels[s] = rr
                have_slot[s] = True
                b = q * QB + sq // 4
                j = sq % 4
                for t, oi in enumerate(idxs):
                    prow = SLOT * j + t
                    hsd[di, q] = heads[oi]
                    tsd[di, q] = tails[oi]
                    scat[di, q] = prow * (QB + 1) + (b - q * QB)
                    ub[prow, b * K:(b + 1) * K] = U[rr]
                    placement.append((int(oi), b, prow))
                    di += 1
            assert di == 128
        routed.append(dict(slot_rels=slot_rels, have_slot=have_slot, hsd=hsd,
                           tsd=tsd, scat=scat, ub=ub, placement=placement))
    return routed, S, NBLK


# ---------------------------------------------------------------------------
# Device program
# ---------------------------------------------------------------------------
def _build_program(S, NBLK):
    nc = bass.Bass("TRN2", target_bir_lowering=False, debug=False)

    # slot-ordered relation table, stored d-major [d, slot, col]
    xtc = nc.dram_tensor("xtc", [DAP, S, NW], FP8, kind="ExternalInput")
    ent = nc.dram_tensor("ent", [NENT, D], F32, kind="ExternalInput")
    hsd = nc.dram_tensor("hsd", [128, DCOL], I32, kind="ExternalInput")
    tsd = nc.dram_tensor("tsd", [128, DCOL], I32, kind="ExternalInput")
    scat = nc.dram_tensor("scat", [128, DCOL], I32, kind="ExternalInput")
    ubt = nc.dram_tensor("ubt", [128, NBLK * K], F32, kind="ExternalInput")
    pred_t = nc.dram_tensor("pred_t", [128, NBLK], F32, kind="ExternalOutput")
    gpre = nc.dram_tensor("gpre", [128, NBLK * K], BF16, kind="ExternalOutput")

    NCH = S // CH  # XT fetch chunks (one chunk covers 4 blocks)

    with tile.TileContext(nc) as tc, ExitStack() as ctx:
        const_pool = ctx.enter_context(tc.tile_pool(name="const", bufs=1))
        dense_pool = ctx.enter_context(tc.tile_pool(name="dense", bufs=1))
        dram_pool = ctx.enter_context(tc.tile_pool(name="bounce", bufs=1,
                                                   space="DRAM"))
        ep_pool = ctx.enter_context(tc.tile_pool(name="epall", bufs=1))
        e1t_pool = ctx.enter_context(tc.tile_pool(name="e1t", bufs=4))
        xt_pool = ctx.enter_context(tc.tile_pool(name="xtrows", bufs=1))
        pbf_pool = ctx.enter_context(tc.tile_pool(name="pbf", bufs=4))
        tmp_pool = ctx.enter_context(tc.tile_pool(name="tmp", bufs=4))
        acc_pool = ctx.enter_context(tc.tile_pool(name="acc", bufs=1))
        psum_p = ctx.enter_context(tc.tile_pool(name="pacc", bufs=2, space="PSUM"))
        psum_t = ctx.enter_context(tc.tile_pool(name="ptrans", bufs=3, space="PSUM"))

        ident = const_pool.tile([128, 128], BF16)
        make_identity(nc, ident[:])

        hsd_t = const_pool.tile([128, DCOL], I32)
        nc.sync.dma_start(hsd_t[:], hsd[:])
        tsd_t = const_pool.tile([128, DCOL], I32)
        nc.sync.dma_start(tsd_t[:], tsd[:])
        scat_t = const_pool.tile([128, DCOL], I32)
        nc.sync.dma_start(scat_t[:], scat[:])
        ub_t = const_pool.tile([128, NBLK * K], F32)
        nc.sync.dma_start(ub_t[:], ubt[:])

        # --- dense entity gathers (SWDGE indirect, f32 -> bf16 cast), then
        # scatters into padded slot order via a DRAM bounce.  Phase-grouped:
        # all gathers emit before the first scatter's sem-wait, so the Pool
        # sequencer never head-of-line-blocks later gathers ---
        QB = NBLK // DCOL
        # one bounce tile + one SBUF staging tile per dense column/quarter:
        # scatter q waits only column q's two gathers, and read-back q waits
        # only scatter q (shared tiles made every op wait on everything)
        bounces = []
        e12s = []
        for c in range(DCOL):
            bounces.append(dram_pool.tile([128 * (QB + 1), RW], BF16,
                                          name=f"bounce{c}", tag=f"bounce_{c}"))
            e12 = dense_pool.tile([128, RW], BF16, tag=f"e12_{c}")
            nc.vector.memset(e12[:], 0.0)
            nc.vector.memset(e12[:, D:D + 1], 1.0)
            nc.vector.memset(e12[:, E2OFF + D:E2OFF + D + 1], 1.0)
            e12s.append(e12)
        # interleave per column: the SWDGE ring drains FIFO, so scatter q's
        # descriptors must enter the ring right after column q's gathers --
        # otherwise they sit behind ALL gather descriptors and the first
        # read-back waits for the whole ~25us drain
        for c in range(DCOL):
            nc.gpsimd.indirect_dma_start(
                out=e12s[c][:, 0:D], out_offset=None, in_=ent[:, :],
                in_offset=IndirectOffsetOnAxis(ap=hsd_t[:, c:c + 1], axis=0))
            nc.gpsimd.indirect_dma_start(
                out=e12s[c][:, E2OFF:E2OFF + D], out_offset=None, in_=ent[:, :],
                in_offset=IndirectOffsetOnAxis(ap=tsd_t[:, c:c + 1], axis=0))
            nc.gpsimd.indirect_dma_start(
                out=bounces[c][:, :],
                out_offset=IndirectOffsetOnAxis(ap=scat_t[:, c:c + 1], axis=0),
                in_=e12s[c][:, :], in_offset=None)

        # --- padded slot-order rows back to SBUF + XT streaming, scheduled
        # around the HWDGE ring FIFOs: the rings drain in issue order, so
        # ep0/ep1 (which gate the first blocks) go at the HEAD of the scalar
        # ring, ahead of the odd XT chunks; ep2/ep3 gate late blocks that
        # are gather-drain-bound anyway, so they ride the sync ring after
        # the even chunks.  Partition-major bounce -> per-partition runs of
        # QB*RW*2 contiguous bytes (big spread descriptors) ---
        ep_all = ep_pool.tile([128, NBLK * RW], BF16)

        def emit_ep(c, eng):
            bv = bounces[c][:, :].rearrange("(p b) x -> p b x", b=QB + 1)
            eng.dma_start(
                ep_all[:, c * QB * RW:(c + 1) * QB * RW]
                .rearrange("p (b x) -> p b x", x=RW),
                bv[:, 0:QB, :])

        emit_ep(0, nc.scalar)
        emit_ep(1, nc.scalar)

        xt_tiles = []
        xt_engines = [nc.sync, nc.scalar]
        for g in range(NCH):
            xtt = xt_pool.tile([DAP, CH * NW], FP8, tag=f"xt{g}")
            xt_engines[g % 2].dma_start(xtt[:], xtc[:, CH * g:CH * (g + 1), :])
            xt_tiles.append(xtt)

        emit_ep(2, nc.sync)
        emit_ep(3, nc.sync)

        # bf16 g_pre accumulator: the tanh is saturated (g_pre ~ 1e3, sigma
        # ~15) so bf16 is ample, and the 16-bit reduce output keeps DVE in
        # its 2x mode
        gpre_t = acc_pool.tile([128, NBLK * K], BF16)

        # transpose the augmented heads rows -> e1~^T [101, 128], hoisted two
        # blocks ahead of the slot matmuls so the PE never waits on the
        # PSUM->SBUF cast of its own block's weights
        e1ts = {}

        def emit_tr(bb):
            tp = psum_t.tile([DA, 128], BF16, tag="tp")
            nc.tensor.transpose(out=tp[:], in_=ep_all[:, bb * RW:bb * RW + DA],
                                identity=ident[:])
            e1t = e1t_pool.tile([DA, 128], FP8, tag="e1t")
            nc.scalar.copy(e1t[:], tp[:])
            e1ts[bb] = e1t

        emit_tr(0)
        if NBLK > 1:
            emit_tr(1)

        # process blocks in PAIRS sharing one bank-aligned 2-bank PSUM tile
        # (block A at f32 cols 0:408, block B at 512:920, so each block's
        # matmul output stays inside one bank) -- one ScalarE copy and one
        # DVE multiply+reduce per pair halves the per-block fixed costs
        PW = 512  # f32 columns per pair half (2KB = one PSUM bank)
        epv2 = ep_all[:].rearrange("p (b x) -> p b x", x=RW)
        for b0 in range(0, NBLK, 2):
            pacc = psum_p.tile([128, 2 * PW], F32)
            for b in (b0, b0 + 1):
                if b + 2 < NBLK:
                    emit_tr(b + 2)
                e1t = e1ts.pop(b)
                poff = (b - b0) * PW
                xtt = xt_tiles[b // 4]
                xoff = (b % 4) * 4 * NW
                for j in range(4):
                    nc.tensor.matmul(
                        out=pacc[SLOT * j:SLOT * (j + 1), poff:poff + NW],
                        lhsT=e1t[:, SLOT * j:SLOT * (j + 1)],
                        rhs=xtt[0:DA, xoff + j * NW:xoff + (j + 1) * NW],
                        start=True, stop=True,
                        tile_position=(0, SLOT * j),
                    )

            # P to bf16 on ScalarE (closer to PSUM), then the segmented
            # multiply+reduce runs on DVE in 2x bf16 mode
            pbf = pbf_pool.tile([128, 2 * NW], BF16)
            nc.scalar.copy(
                pbf[:].rearrange("p (t x) -> p t x", t=2),
                pacc[:].rearrange("p (t w) -> p t w", t=2)[:, :, 0:NW])
            tmp = tmp_pool.tile([128, 2 * NW], BF16)
            nc.vector.tensor_tensor(
                out=tmp[:].rearrange("p (t k j) -> p t k j", t=2, k=K),
                in0=pbf[:].rearrange("p (t k j) -> p t k j", t=2, k=K),
                in1=epv2[:, b0:b0 + 2, E2OFF:E2OFF + DJ]
                    .unsqueeze(2).broadcast_to([128, 2, K, DJ]),
                op=mybir.AluOpType.mult,
            )
            with nc.allow_low_precision(reason="tanh-saturated g_pre"):
                nc.vector.reduce_sum(
                    out=gpre_t[:, K * b0:K * (b0 + 2)],
                    in_=tmp[:].rearrange("p (t k j) -> p t k j", t=2, k=K),
                    axis=mybir.AxisListType.X,
                )

        # --- batched tail: tanh, u-weighting, k-reduce, sigmoid ---
        th = const_pool.tile([128, NBLK * K], F32)
        nc.scalar.activation(th[:], gpre_t[:],
                             mybir.ActivationFunctionType.Tanh)
        scr = const_pool.tile([128, NBLK * K], F32)
        nc.vector.tensor_tensor(out=scr[:], in0=th[:], in1=ub_t[:],
                                op=mybir.AluOpType.mult)
        sco = const_pool.tile([128, NBLK], F32)
        nc.vector.reduce_sum(
            out=sco[:], in_=scr[:].rearrange("p (b k) -> p b k", k=K),
            axis=mybir.AxisListType.X)
        pred_sb = const_pool.tile([128, NBLK], F32)
        nc.scalar.activation(pred_sb[:], sco[:],
                             mybir.ActivationFunctionType.Sigmoid)
        nc.sync.dma_start(pred_t[:], pred_sb[:])
        nc.sync.dma_start(gpre[:], gpre_t[:])

    return nc


_PROGRAM_CACHE = {}


def _get_program(S, NBLK):
    key = (S, NBLK)
    if key not in _PROGRAM_CACHE:
        _PROGRAM_CACHE[key] = _build_program(S, NBLK)
    return _PROGRAM_CACHE[key]


# ---------------------------------------------------------------------------
# Entry point
# ---------------------------------------------------------------------------
def _run(inputs, trace=False, tmpdir=None, trace_cores=None):
    from concourse.bass_utils import run_bass_kernel_spmd

    heads = np.asarray(inputs["heads"]).astype(np.int64)
    tails = np.asarray(inputs["tails"]).astype(np.int64)
    relations = np.asarray(inputs["relations"]).astype(np.int64)
    ent = np.ascontiguousarray(np.asarray(inputs["entity_embedding"], np.float32))
    W = np.asarray(inputs["W"], np.float32)
    V = np.asarray(inputs["V"], np.float32)
    Bp = np.asarray(inputs["Bp"], np.float32)
    U = np.asarray(inputs["U"], np.float32)

    XTb = _build_xt(W, V, Bp)
    routed, S, NBLK = _route(heads, tails, relations, U)

    nc = _get_program(S, NBLK)

    in_maps = []
    for c in range(NCORES):
        r = routed[c]
        xtg = XTb[r["slot_rels"]]                   # [S, DA, NW] bf16
        xtg[~r["have_slot"]] = 0
        in_maps.append({
            "xtc": np.ascontiguousarray(xtg.transpose(1, 0, 2)),
            "ent": ent,
            "hsd": r["hsd"],
            "tsd": r["tsd"],
            "scat": r["scat"],
            "ubt": r["ub"],
        })

    kwargs = {}
    if trace:
        kwargs.update(trace=True, tmpdir=tmpdir)
        if trace_cores is not None:
            kwargs.update(trace_cores=trace_cores)
    res = run_bass_kernel_spmd(nc, in_maps, core_ids=list(range(NCORES)), **kwargs)

    pred = np.zeros(B, np.float32)
    for c in range(NCORES):
        pt = res.results[c]["pred_t"]  # [128, NBLK]
        for oi, b, p in routed[c]["placement"]:
            pred[oi] = pt[p, b]
    return pred, routed, res


def kernel(**inputs):
    pred, _, _ = _run(inputs)
    return pred


# revision 45
# speedup vs baseline: 1.0146x; 1.0146x over previous
"""Neural Tensor Network (NTN) scoring kernel for Trainium2 (Bass/Tile).

score_k(e1, e2, r) = u_k . tanh( e1^T W[r,k] e2 + v_k . [e1;e2] + b_k )
pred = sigmoid( sum_k score_k )

Strategy (v2)
-------------
Host: group the batch by relation id, pack each group into 32-item slots
(PE column-strip granularity), and greedily balance the slots across the
8 cores.  All per-relation parameters except u are folded into one
augmented bf16 table XTb[r] of shape [101, 4*102] such that with
e1~ = [e1; 1]:

    P[k*102 + j] = (e1^T W_k)[j] + v_k^b[j]     (j < 100)
    P[k*102+100] = v_k^a . e1 + b_k
    P[k*102+101] = 0                             (alignment pad)

so with e2~ = [e2; 1; 0]:  g_pre_k = sum_j P[k*102+j] * e2~[j]
and  pred = sigmoid( sum_k u_k * tanh(g_pre_k) ).  u stays f32 in a
separate per-lane table (zeros on padding lanes, which also neutralises
garbage rows).

Device (one SPMD program on 8 cores):
  * entity rows are gathered on-device (SWDGE indirect, f32->bf16 cast),
    scattered into padded slot order through a DRAM bounce buffer, and
    read back with a rearranged AP,
  * the core's whole slot-ordered XT shard streams into SBUF in 4-slot
    chunks (3264B per-partition descriptor runs -- kept under 4KB so the
    HWDGE spreads them over all 16 SDMA engines; bigger runs all pin to
    one engine at ~27GB/s, which was the v1 bottleneck),
  * per 128-row block: PE transposes the e1~ rows (bf16), four matmuls
    (one per 32-item slot, packed into the four column strips of one
    PSUM tile) produce P, VectorE does the segmented e2~ multiply+reduce,
  * one batched tanh / u-multiply / k-reduce / sigmoid tail.
"""

import sys
from contextlib import ExitStack

for _p in ("/opt/trn_rl_repo", "/opt/trn_rl_repo/concourse"):
    if _p not in sys.path:
        sys.path.insert(0, _p)

import numpy as np  # noqa: E402
import ml_dtypes  # noqa: E402

import concourse.bass as bass  # noqa: E402
import concourse.mybir as mybir  # noqa: E402
import concourse.tile as tile  # noqa: E402
from concourse.bass import IndirectOffsetOnAxis  # noqa: E402
from concourse.masks import make_identity  # noqa: E402

F32 = mybir.dt.float32
BF16 = mybir.dt.bfloat16
FP8 = mybir.dt.float8e4
I32 = mybir.dt.int32
BF16_NP = ml_dtypes.bfloat16
FP8_NP = ml_dtypes.float8_e4m3

B = 4096
D = 100
K = 4
NREL = 1000
NENT = 100000
NCORES = 8
DA = D + 1           # augmented contraction dim (e1; 1)
DAP = 104            # DA padded to a multiple of 8: DMAs whose per-partition
                     # descriptor count is not a multiple of 8 all land on ONE
                     # SDMA engine (~27GB/s); 104 rows spread over 13 engines
DJ = DA + 1          # 102: padded e2~ segment (e1^T W | bias | 0)
NW = K * DJ          # 408 folded W/V/B columns (bf16)
SLOT = 32            # items per slot (PE col-strip granularity)
CH = 16              # slots per XT fetch chunk (4 blocks)
CAP = 512            # per-core item capacity (hsd layout is [128, 4])
DCOL = CAP // 128
# ep_all row layout (bf16): [e1 (0:100) | 1 | pad3 | e2 (104:204) | 1 | pad3]
RW = 208
E2OFF = 104          # e2~ segment start (4B aligned), 102 cols used


# ---------------------------------------------------------------------------
# Walrus on this toolchain rejects instructions carrying more than one
# sync-wait command. After Tile schedules, move any excess waits onto
# freshly inserted same-engine nops placed directly before the instruction
# (engines execute their stream in order, so semantics are unchanged).
# ---------------------------------------------------------------------------
_WAIT_LIMIT = 1
_split_counter = [0]


def _split_excess_waits(nc):
    for f in nc.m.functions:
        for blk in f.blocks:
            il = blk.instructions
            k = 0
            while k < len(il):
                inst = il[k]
                si = inst.sync_info
                if si is not None and si.on_wait and len(si.on_wait) > _WAIT_LIMIT:
                    waits = list(si.on_wait)
                    excess = waits[:-_WAIT_LIMIT]
                    del si.on_wait[:-_WAIT_LIMIT]
                    for w in excess:
                        _split_counter[0] += 1
                        nop = mybir.InstNoOp(
                            name=f"I-waitsplit-{_split_counter[0]}", ins=[], outs=[])
                        nop.engine = inst.engine
                        nop.sync_info = mybir.SyncInfo(on_wait=[w], on_update=[])
                        nc.register_instruction(nop, overwrite=True)
                        il.insert(k, nop)
                        k += 1
                k += 1


_orig_tile_exit = tile.TileContext.__exit__


def _patched_tile_exit(self, exc_type, exc, tb):
    r = _orig_tile_exit(self, exc_type, exc, tb)
    if exc_type is None:
        _split_excess_waits(self.nc)
    return r


if getattr(tile.TileContext, "_ant_wait_split_patch", False) is False:
    tile.TileContext.__exit__ = _patched_tile_exit
    tile.TileContext._ant_wait_split_patch = True


# ---------------------------------------------------------------------------
# Host-side preparation
# ---------------------------------------------------------------------------
def _build_xt(W, V, Bp):
    """Fold W/V/Bp into the augmented relation table XTb [NREL, DAP, NW] fp8.

    fp8e4m3 keeps ~2 decimal digits; the bilinear scores are ~1e3 with sigma
    ~15 so tanh is saturated far beyond fp8's error, and u (the only factor
    the final sigmoid is sensitive to) stays f32 in a separate table.
    """
    core = np.zeros((NREL, DAP, K, DJ), np.float32)
    core[:, :D, :, :D] = W.transpose(0, 2, 1, 3)          # [r, d, k, e]
    core[:, D, :, :D] = V[:, :, D:]                        # v^b
    core[:, :D, :, D] = V[:, :, :D].transpose(0, 2, 1)     # v^a
    core[:, D, :, D] = Bp
    return core.reshape(NREL, DAP, NW).astype(FP8_NP)


def _route(heads, tails, relations, U):
    """Group by relation, pack 32-item slots, balance slots across cores."""
    order = np.argsort(relations, kind="stable")
    rels = relations[order]
    # global slot list: (relation id, item indices)
    slots = []
    i = 0
    n = len(order)
    while i < n:
        j = i
        while j < n and rels[j] == rels[i]:
            j += 1
        for a in range(i, j, SLOT):
            slots.append((int(rels[i]), order[a:min(a + SLOT, j)]))
        i = j

    # greedy balance: total items exactly fills NCORES*CAP, so items are the
    # binding constraint -- place big slots first into the core with most
    # remaining capacity (ties: fewest slots), which also balances slot counts
    core_slots = [[] for _ in range(NCORES)]
    core_items = [0] * NCORES
    for s in sorted(slots, key=lambda s: -len(s[1])):
        c = min(range(NCORES),
                key=lambda c: (core_items[c] + len(s[1]) > CAP,
                               -(CAP - core_items[c]), len(core_slots[c])))
        if core_items[c] + len(s[1]) > CAP:
            raise RuntimeError("slot does not fit on any core")
        core_slots[c].append(s)
        core_items[c] += len(s[1])

    # split each core's slot list into 4 quarters of EXACTLY 128 items
    # (splitting the straddling slot keeps this always feasible).  Quarter q
    # becomes dense column q, and its blocks get their own bounce tile, so
    # scatter q depends only on column q's gathers and read-back q only on
    # scatter q.
    core_quarters = []
    for cs in core_slots:
        # deal slots into the 4 quarters balancing both items (exactly 128
        # each; slots split across quarters when needed) and slot counts
        quarters = [[] for _ in range(DCOL)]
        items = [0] * DCOL
        for rr, idxs in sorted(cs, key=lambda s: -len(s[1])):
            idxs = list(idxs)
            while idxs:
                q = min(range(DCOL),
                        key=lambda q: (items[q] >= 128, len(quarters[q]),
                                       -(128 - items[q])))
                assert items[q] < 128
                take = min(128 - items[q], len(idxs))
                quarters[q].append((rr, np.asarray(idxs[:take])))
                items[q] += take
                idxs = idxs[take:]
        assert all(n == 128 for n in items)
        core_quarters.append(quarters)

    QB = max((len(q) + 3) // 4
             for quarters in core_quarters for q in quarters)
    NBLK = DCOL * QB
    S = NBLK * 4

    routed = []
    for c in range(NCORES):
        quarters = core_quarters[c]
        slot_rels = np.zeros(S, np.int64)
        have_slot = np.zeros(S, np.bool_)
        # dense-order entity ids + dense -> padded-slot scatter targets.
        # bounce_q is partition-major [128 lanes, QB+1 blocks]: row index
        # p*(QB+1) + local block, read back as 128 contiguous runs
        hsd = np.zeros((128, DCOL), np.int32)
        tsd = np.zeros((128, DCOL), np.int32)
        scat = np.zeros((128, DCOL), np.int32)
        for p in range(128):
            for cc in range(DCOL):
                scat[p, cc] = p * (QB + 1) + QB   # dump col for unused
        ub = np.zeros((128, NBLK * K), np.float32)
        placement = []  # (orig batch index, block, partition row)
        for q, qslots in enumerate(quarters):
            di = 0
            for sq, (rr, idxs) in enumerate(qslots):
                s = q * QB * 4 + sq
                slot_rels[s] = rr
                have_slot[s] = True
                b = q * QB + sq // 4
                j = sq % 4
                for t, oi in enumerate(idxs):
                    prow = SLOT * j + t
                    hsd[di, q] = heads[oi]
                    tsd[di, q] = tails[oi]
                    scat[di, q] = prow * (QB + 1) + (b - q * QB)
                    ub[prow, b * K:(b + 1) * K] = U[rr]
                    placement.append((int(oi), b, prow))
                    di += 1
            assert di == 128
        routed.append(dict(slot_rels=slot_rels, have_slot=have_slot, hsd=hsd,
                           tsd=tsd, scat=scat, ub=ub, placement=placement))
    return routed, S, NBLK


# ---------------------------------------------------------------------------
# Device program
# ---------------------------------------------------------------------------
def _build_program(S, NBLK):
    nc = bass.Bass("TRN2", target_bir_lowering=False, debug=False)

    # slot-ordered relation table, stored d-major [d, slot, col]
    xtc = nc.dram_tensor("xtc", [DAP, S, NW], FP8, kind="ExternalInput")
    ent = nc.dram_tensor("ent", [NENT, D], F32, kind="ExternalInput")
    hsd = nc.dram_tensor("hsd", [128, DCOL], I32, kind="ExternalInput")
    tsd = nc.dram_tensor("tsd", [128, DCOL], I32, kind="ExternalInput")
    scat = nc.dram_tensor("scat", [128, DCOL], I32, kind="ExternalInput")
    ubt = nc.dram_tensor("ubt", [128, NBLK * K], F32, kind="ExternalInput")
    pred_t = nc.dram_tensor("pred_t", [128, NBLK], F32, kind="ExternalOutput")
    gpre = nc.dram_tensor("gpre", [128, NBLK * K], BF16, kind="ExternalOutput")

    NCH = S // CH  # XT fetch chunks (one chunk covers 4 blocks)

    with tile.TileContext(nc) as tc, ExitStack() as ctx:
        const_pool = ctx.enter_context(tc.tile_pool(name="const", bufs=1))
        dense_pool = ctx.enter_context(tc.tile_pool(name="dense", bufs=1))
        dram_pool = ctx.enter_context(tc.tile_pool(name="bounce", bufs=1,
                                                   space="DRAM"))
        ep_pool = ctx.enter_context(tc.tile_pool(name="epall", bufs=1))
        e1t_pool = ctx.enter_context(tc.tile_pool(name="e1t", bufs=4))
        xt_pool = ctx.enter_context(tc.tile_pool(name="xtrows", bufs=1))
        pbf_pool = ctx.enter_context(tc.tile_pool(name="pbf", bufs=4))
        tmp_pool = ctx.enter_context(tc.tile_pool(name="tmp", bufs=4))
        acc_pool = ctx.enter_context(tc.tile_pool(name="acc", bufs=1))
        psum_p = ctx.enter_context(tc.tile_pool(name="pacc", bufs=4, space="PSUM"))
        psum_t = ctx.enter_context(tc.tile_pool(name="ptrans", bufs=3, space="PSUM"))

        ident = const_pool.tile([128, 128], BF16)
        make_identity(nc, ident[:])

        hsd_t = const_pool.tile([128, DCOL], I32)
        nc.sync.dma_start(hsd_t[:], hsd[:])
        tsd_t = const_pool.tile([128, DCOL], I32)
        nc.sync.dma_start(tsd_t[:], tsd[:])
        scat_t = const_pool.tile([128, DCOL], I32)
        nc.sync.dma_start(scat_t[:], scat[:])
        ub_t = const_pool.tile([128, NBLK * K], F32)
        nc.sync.dma_start(ub_t[:], ubt[:])

        # --- dense entity gathers (SWDGE indirect, f32 -> bf16 cast), then
        # scatters into padded slot order via a DRAM bounce.  Phase-grouped:
        # all gathers emit before the first scatter's sem-wait, so the Pool
        # sequencer never head-of-line-blocks later gathers ---
        QB = NBLK // DCOL
        # one bounce tile + one SBUF staging tile per dense column/quarter:
        # scatter q waits only column q's two gathers, and read-back q waits
        # only scatter q (shared tiles made every op wait on everything)
        bounces = []
        e12s = []
        for c in range(DCOL):
            bounces.append(dram_pool.tile([128 * (QB + 1), RW], BF16,
                                          name=f"bounce{c}", tag=f"bounce_{c}"))
            e12 = dense_pool.tile([128, RW], BF16, tag=f"e12_{c}")
            nc.vector.memset(e12[:], 0.0)
            nc.vector.memset(e12[:, D:D + 1], 1.0)
            nc.vector.memset(e12[:, E2OFF + D:E2OFF + D + 1], 1.0)
            e12s.append(e12)
        # interleave per column: the SWDGE ring drains FIFO, so scatter q's
        # descriptors must enter the ring right after column q's gathers --
        # otherwise they sit behind ALL gather descriptors and the first
        # read-back waits for the whole ~25us drain
        for c in range(DCOL):
            nc.gpsimd.indirect_dma_start(
                out=e12s[c][:, 0:D], out_offset=None, in_=ent[:, :],
                in_offset=IndirectOffsetOnAxis(ap=hsd_t[:, c:c + 1], axis=0))
            nc.gpsimd.indirect_dma_start(
                out=e12s[c][:, E2OFF:E2OFF + D], out_offset=None, in_=ent[:, :],
                in_offset=IndirectOffsetOnAxis(ap=tsd_t[:, c:c + 1], axis=0))
            nc.gpsimd.indirect_dma_start(
                out=bounces[c][:, :],
                out_offset=IndirectOffsetOnAxis(ap=scat_t[:, c:c + 1], axis=0),
                in_=e12s[c][:, :], in_offset=None)

        # --- padded slot-order rows back to SBUF + XT streaming, scheduled
        # around the HWDGE ring FIFOs: the rings drain in issue order, so
        # ep0/ep1 (which gate the first blocks) go at the HEAD of the scalar
        # ring, ahead of the odd XT chunks; ep2/ep3 gate late blocks that
        # are gather-drain-bound anyway, so they ride the sync ring after
        # the even chunks.  Partition-major bounce -> per-partition runs of
        # QB*RW*2 contiguous bytes (big spread descriptors) ---
        ep_all = ep_pool.tile([128, NBLK * RW], BF16)

        def emit_ep(c, eng):
            bv = bounces[c][:, :].rearrange("(p b) x -> p b x", b=QB + 1)
            eng.dma_start(
                ep_all[:, c * QB * RW:(c + 1) * QB * RW]
                .rearrange("p (b x) -> p b x", x=RW),
                bv[:, 0:QB, :])

        emit_ep(0, nc.scalar)
        emit_ep(1, nc.scalar)

        xt_tiles = []
        xt_engines = [nc.sync, nc.scalar]
        for g in range(NCH):
            xtt = xt_pool.tile([DAP, CH * NW], FP8, tag=f"xt{g}")
            xt_engines[g % 2].dma_start(xtt[:], xtc[:, CH * g:CH * (g + 1), :])
            xt_tiles.append(xtt)

        emit_ep(2, nc.sync)
        emit_ep(3, nc.sync)

        # bf16 g_pre accumulator: the tanh is saturated (g_pre ~ 1e3, sigma
        # ~15) so bf16 is ample, and the 16-bit reduce output keeps DVE in
        # its 2x mode
        gpre_t = acc_pool.tile([128, NBLK * K], BF16)

        # transpose the augmented heads rows -> e1~^T [101, 128], hoisted two
        # blocks ahead of the slot matmuls so the PE never waits on the
        # PSUM->SBUF cast of its own block's weights
        e1ts = {}

        def emit_tr(bb):
            tp = psum_t.tile([DA, 128], BF16, tag="tp")
            nc.tensor.transpose(out=tp[:], in_=ep_all[:, bb * RW:bb * RW + DA],
                                identity=ident[:])
            e1t = e1t_pool.tile([DA, 128], FP8, tag="e1t")
            nc.scalar.copy(e1t[:], tp[:])
            e1ts[bb] = e1t

        emit_tr(0)
        if NBLK > 1:
            emit_tr(1)

        for b in range(NBLK):
            if b + 2 < NBLK:
                emit_tr(b + 2)
            e1t = e1ts.pop(b)

            # four slot matmuls into the four column strips of one PSUM tile
            pacc = psum_p.tile([128, NW], F32)
            xtt = xt_tiles[b // 4]
            xoff = (b % 4) * 4 * NW
            for j in range(4):
                nc.tensor.matmul(
                    out=pacc[SLOT * j:SLOT * (j + 1), 0:NW],
                    lhsT=e1t[:, SLOT * j:SLOT * (j + 1)],
                    rhs=xtt[0:DA, xoff + j * NW:xoff + (j + 1) * NW],
                    start=True, stop=True,
                    tile_position=(0, SLOT * j),
                )

            # P to bf16 on ScalarE (closer to PSUM), then the segmented
            # multiply+reduce runs on DVE in 2x bf16 mode
            pbf = pbf_pool.tile([128, NW], BF16)
            nc.scalar.copy(pbf[:], pacc[:])
            tmp = tmp_pool.tile([128, NW], BF16)
            nc.vector.tensor_tensor(
                out=tmp[:].rearrange("p (k j) -> p k j", k=K),
                in0=pbf[:].rearrange("p (k j) -> p k j", k=K),
                in1=ep_all[:, b * RW + E2OFF:b * RW + E2OFF + DJ]
                    .unsqueeze(1).broadcast_to([128, K, DJ]),
                op=mybir.AluOpType.mult,
            )
            with nc.allow_low_precision(reason="tanh-saturated g_pre"):
                nc.vector.reduce_sum(
                    out=gpre_t[:, K * b:K * (b + 1)],
                    in_=tmp[:].rearrange("p (k j) -> p k j", k=K),
                    axis=mybir.AxisListType.X,
                )

        # --- batched tail: tanh, u-weighting, k-reduce, sigmoid ---
        th = const_pool.tile([128, NBLK * K], F32)
        nc.scalar.activation(th[:], gpre_t[:],
                             mybir.ActivationFunctionType.Tanh)
        scr = const_pool.tile([128, NBLK * K], F32)
        nc.vector.tensor_tensor(out=scr[:], in0=th[:], in1=ub_t[:],
                                op=mybir.AluOpType.mult)
        sco = const_pool.tile([128, NBLK], F32)
        nc.vector.reduce_sum(
            out=sco[:], in_=scr[:].rearrange("p (b k) -> p b k", k=K),
            axis=mybir.AxisListType.X)
        pred_sb = const_pool.tile([128, NBLK], F32)
        nc.scalar.activation(pred_sb[:], sco[:],
                             mybir.ActivationFunctionType.Sigmoid)
        nc.sync.dma_start(pred_t[:], pred_sb[:])
        nc.sync.dma_start(gpre[:], gpre_t[:])

    return nc


_PROGRAM_CACHE = {}


def _get_program(S, NBLK):
    key = (S, NBLK)
    if key not in _PROGRAM_CACHE:
        _PROGRAM_CACHE[key] = _build_program(S, NBLK)
    return _PROGRAM_CACHE[key]


# ---------------------------------------------------------------------------
# Entry point
# ---------------------------------------------------------------------------
def _run(inputs, trace=False, tmpdir=None, trace_cores=None):
    from concourse.bass_utils import run_bass_kernel_spmd

    heads = np.asarray(inputs["heads"]).astype(np.int64)
    tails = np.asarray(inputs["tails"]).astype(np.int64)
    relations = np.asarray(inputs["relations"]).astype(np.int64)
    ent = np.ascontiguousarray(np.asarray(inputs["entity_embedding"], np.float32))
    W = np.asarray(inputs["W"], np.float32)
    V = np.asarray(inputs["V"], np.float32)
    Bp = np.asarray(inputs["Bp"], np.float32)
    U = np.asarray(inputs["U"], np.float32)

    XTb = _build_xt(W, V, Bp)
    routed, S, NBLK = _route(heads, tails, relations, U)

    nc = _get_program(S, NBLK)

    in_maps = []
    for c in range(NCORES):
        r = routed[c]
        xtg = XTb[r["slot_rels"]]                   # [S, DA, NW] bf16
        xtg[~r["have_slot"]] = 0
        in_maps.append({
            "xtc": np.ascontiguousarray(xtg.transpose(1, 0, 2)),
            "ent": ent,
            "hsd": r["hsd"],
            "tsd": r["tsd"],
            "scat": r["scat"],
            "ubt": r["ub"],
        })

    kwargs = {}
    if trace:
        kwargs.update(trace=True, tmpdir=tmpdir)
        if trace_cores is not None:
            kwargs.update(trace_cores=trace_cores)
    res = run_bass_kernel_spmd(nc, in_maps, core_ids=list(range(NCORES)), **kwargs)

    pred = np.zeros(B, np.float32)
    for c in range(NCORES):
        pt = res.results[c]["pred_t"]  # [128, NBLK]
        for oi, b, p in routed[c]["placement"]:
            pred[oi] = pt[p, b]
    return pred, routed, res


def kernel(**inputs):
    pred, _, _ = _run(inputs)
    return pred


# revision 47
# speedup vs baseline: 1.0512x; 1.0360x over previous
"""Neural Tensor Network (NTN) scoring kernel for Trainium2 (Bass/Tile).

score_k(e1, e2, r) = u_k . tanh( e1^T W[r,k] e2 + v_k . [e1;e2] + b_k )
pred = sigmoid( sum_k score_k )

Strategy (v2)
-------------
Host: group the batch by relation id, pack each group into 32-item slots
(PE column-strip granularity), and greedily balance the slots across the
8 cores.  All per-relation parameters except u are folded into one
augmented bf16 table XTb[r] of shape [101, 4*102] such that with
e1~ = [e1; 1]:

    P[k*102 + j] = (e1^T W_k)[j] + v_k^b[j]     (j < 100)
    P[k*102+100] = v_k^a . e1 + b_k
    P[k*102+101] = 0                             (alignment pad)

so with e2~ = [e2; 1; 0]:  g_pre_k = sum_j P[k*102+j] * e2~[j]
and  pred = sigmoid( sum_k u_k * tanh(g_pre_k) ).  u stays f32 in a
separate per-lane table (zeros on padding lanes, which also neutralises
garbage rows).

Device (one SPMD program on 8 cores):
  * entity rows are gathered on-device (SWDGE indirect, f32->bf16 cast),
    scattered into padded slot order through a DRAM bounce buffer, and
    read back with a rearranged AP,
  * the core's whole slot-ordered XT shard streams into SBUF in 4-slot
    chunks (3264B per-partition descriptor runs -- kept under 4KB so the
    HWDGE spreads them over all 16 SDMA engines; bigger runs all pin to
    one engine at ~27GB/s, which was the v1 bottleneck),
  * per 128-row block: PE transposes the e1~ rows (bf16), four matmuls
    (one per 32-item slot, packed into the four column strips of one
    PSUM tile) produce P, VectorE does the segmented e2~ multiply+reduce,
  * one batched tanh / u-multiply / k-reduce / sigmoid tail.
"""

import sys
from contextlib import ExitStack

for _p in ("/opt/trn_rl_repo", "/opt/trn_rl_repo/concourse"):
    if _p not in sys.path:
        sys.path.insert(0, _p)

import numpy as np  # noqa: E402
import ml_dtypes  # noqa: E402

import concourse.bass as bass  # noqa: E402
import concourse.mybir as mybir  # noqa: E402
import concourse.tile as tile  # noqa: E402
from concourse.bass import IndirectOffsetOnAxis  # noqa: E402
from concourse.masks import make_identity  # noqa: E402

F32 = mybir.dt.float32
BF16 = mybir.dt.bfloat16
FP8 = mybir.dt.float8e4
I32 = mybir.dt.int32
BF16_NP = ml_dtypes.bfloat16
FP8_NP = ml_dtypes.float8_e4m3

B = 4096
D = 100
K = 4
NREL = 1000
NENT = 100000
NCORES = 8
DA = D + 1           # augmented contraction dim (e1; 1)
DAP = 104            # DA padded to a multiple of 8: DMAs whose per-partition
                     # descriptor count is not a multiple of 8 all land on ONE
                     # SDMA engine (~27GB/s); 104 rows spread over 13 engines
DJ = DA + 1          # 102: padded e2~ segment (e1^T W | bias | 0)
NW = K * DJ          # 408 folded W/V/B columns (bf16)
SLOT = 32            # items per slot (PE col-strip granularity)
CH = 16              # slots per XT fetch chunk (4 blocks)
CAP = 512            # per-core item capacity (hsd layout is [128, 4])
DCOL = CAP // 128
# ep_all row layout (bf16): [e1 (0:100) | 1 | pad3 | e2 (104:204) | 1 | pad3]
RW = 208
E2OFF = 104          # e2~ segment start (4B aligned), 102 cols used


# ---------------------------------------------------------------------------
# Walrus on this toolchain rejects instructions carrying more than one
# sync-wait command. After Tile schedules, move any excess waits onto
# freshly inserted same-engine nops placed directly before the instruction
# (engines execute their stream in order, so semantics are unchanged).
# ---------------------------------------------------------------------------
_WAIT_LIMIT = 1
_split_counter = [0]


def _split_excess_waits(nc):
    for f in nc.m.functions:
        for blk in f.blocks:
            il = blk.instructions
            k = 0
            while k < len(il):
                inst = il[k]
                si = inst.sync_info
                if si is not None and si.on_wait and len(si.on_wait) > _WAIT_LIMIT:
                    waits = list(si.on_wait)
                    excess = waits[:-_WAIT_LIMIT]
                    del si.on_wait[:-_WAIT_LIMIT]
                    for w in excess:
                        _split_counter[0] += 1
                        nop = mybir.InstNoOp(
                            name=f"I-waitsplit-{_split_counter[0]}", ins=[], outs=[])
                        nop.engine = inst.engine
                        nop.sync_info = mybir.SyncInfo(on_wait=[w], on_update=[])
                        nc.register_instruction(nop, overwrite=True)
                        il.insert(k, nop)
                        k += 1
                k += 1


_orig_tile_exit = tile.TileContext.__exit__


def _patched_tile_exit(self, exc_type, exc, tb):
    r = _orig_tile_exit(self, exc_type, exc, tb)
    if exc_type is None:
        _split_excess_waits(self.nc)
    return r


if getattr(tile.TileContext, "_ant_wait_split_patch", False) is False:
    tile.TileContext.__exit__ = _patched_tile_exit
    tile.TileContext._ant_wait_split_patch = True


# ---------------------------------------------------------------------------
# Host-side preparation
# ---------------------------------------------------------------------------
def _build_xt(W, V, Bp):
    """Fold W/V/Bp into the augmented relation table XTb [NREL, DAP, NW] fp8.

    fp8e4m3 keeps ~2 decimal digits; the bilinear scores are ~1e3 with sigma
    ~15 so tanh is saturated far beyond fp8's error, and u (the only factor
    the final sigmoid is sensitive to) stays f32 in a separate table.
    """
    core = np.zeros((NREL, DAP, K, DJ), np.float32)
    core[:, :D, :, :D] = W.transpose(0, 2, 1, 3)          # [r, d, k, e]
    core[:, D, :, :D] = V[:, :, D:]                        # v^b
    core[:, :D, :, D] = V[:, :, :D].transpose(0, 2, 1)     # v^a
    core[:, D, :, D] = Bp
    return core.reshape(NREL, DAP, NW).astype(FP8_NP)


def _route(heads, tails, relations, U):
    """Group by relation, pack 32-item slots, balance slots across cores."""
    order = np.argsort(relations, kind="stable")
    rels = relations[order]
    # global slot list: (relation id, item indices)
    slots = []
    i = 0
    n = len(order)
    while i < n:
        j = i
        while j < n and rels[j] == rels[i]:
            j += 1
        for a in range(i, j, SLOT):
            slots.append((int(rels[i]), order[a:min(a + SLOT, j)]))
        i = j

    # greedy balance: total items exactly fills NCORES*CAP, so items are the
    # binding constraint -- place big slots first into the core with most
    # remaining capacity (ties: fewest slots), which also balances slot counts
    core_slots = [[] for _ in range(NCORES)]
    core_items = [0] * NCORES
    for s in sorted(slots, key=lambda s: -len(s[1])):
        c = min(range(NCORES),
                key=lambda c: (core_items[c] + len(s[1]) > CAP,
                               -(CAP - core_items[c]), len(core_slots[c])))
        if core_items[c] + len(s[1]) > CAP:
            raise RuntimeError("slot does not fit on any core")
        core_slots[c].append(s)
        core_items[c] += len(s[1])

    # split each core's slot list into 4 quarters of EXACTLY 128 items
    # (splitting the straddling slot keeps this always feasible).  Quarter q
    # becomes dense column q, and its blocks get their own bounce tile, so
    # scatter q depends only on column q's gathers and read-back q only on
    # scatter q.
    core_quarters = []
    for cs in core_slots:
        # deal slots into the 4 quarters balancing both items (exactly 128
        # each; slots split across quarters when needed) and slot counts
        quarters = [[] for _ in range(DCOL)]
        items = [0] * DCOL
        for rr, idxs in sorted(cs, key=lambda s: -len(s[1])):
            idxs = list(idxs)
            while idxs:
                q = min(range(DCOL),
                        key=lambda q: (items[q] >= 128, len(quarters[q]),
                                       -(128 - items[q])))
                assert items[q] < 128
                take = min(128 - items[q], len(idxs))
                quarters[q].append((rr, np.asarray(idxs[:take])))
                items[q] += take
                idxs = idxs[take:]
        assert all(n == 128 for n in items)
        core_quarters.append(quarters)

    QB = max((len(q) + 3) // 4
             for quarters in core_quarters for q in quarters)
    NBLK = DCOL * QB
    S = NBLK * 4

    routed = []
    for c in range(NCORES):
        quarters = core_quarters[c]
        slot_rels = np.zeros(S, np.int64)
        have_slot = np.zeros(S, np.bool_)
        # dense-order entity ids + dense -> padded-slot scatter targets.
        # bounce_q is partition-major [128 lanes, QB+1 blocks]: row index
        # p*(QB+1) + local block, read back as 128 contiguous runs
        hsd = np.zeros((128, DCOL), np.int32)
        tsd = np.zeros((128, DCOL), np.int32)
        scat = np.zeros((128, DCOL), np.int32)
        for p in range(128):
            for cc in range(DCOL):
                scat[p, cc] = p * (QB + 1) + QB   # dump col for unused
        ub = np.zeros((128, NBLK * K), np.float32)
        placement = []  # (orig batch index, block, partition row)
        for q, qslots in enumerate(quarters):
            di = 0
            for sq, (rr, idxs) in enumerate(qslots):
                s = q * QB * 4 + sq
                slot_rels[s] = rr
                have_slot[s] = True
                b = q * QB + sq // 4
                j = sq % 4
                for t, oi in enumerate(idxs):
                    prow = SLOT * j + t
                    hsd[di, q] = heads[oi]
                    tsd[di, q] = tails[oi]
                    scat[di, q] = prow * (QB + 1) + (b - q * QB)
                    ub[prow, b * K:(b + 1) * K] = U[rr]
                    placement.append((int(oi), b, prow))
                    di += 1
            assert di == 128
        routed.append(dict(slot_rels=slot_rels, have_slot=have_slot, hsd=hsd,
                           tsd=tsd, scat=scat, ub=ub, placement=placement))
    return routed, S, NBLK


# ---------------------------------------------------------------------------
# Device program
# ---------------------------------------------------------------------------
def _build_program(S, NBLK):
    nc = bass.Bass("TRN2", target_bir_lowering=False, debug=False)

    # slot-ordered relation table, stored d-major [d, slot, col]
    xtc = nc.dram_tensor("xtc", [DAP, S, NW], FP8, kind="ExternalInput")
    ent = nc.dram_tensor("ent", [NENT, D], F32, kind="ExternalInput")
    hsd = nc.dram_tensor("hsd", [128, DCOL], I32, kind="ExternalInput")
    tsd = nc.dram_tensor("tsd", [128, DCOL], I32, kind="ExternalInput")
    scat = nc.dram_tensor("scat", [128, DCOL], I32, kind="ExternalInput")
    ubt = nc.dram_tensor("ubt", [128, NBLK * K], F32, kind="ExternalInput")
    pred_t = nc.dram_tensor("pred_t", [128, NBLK], F32, kind="ExternalOutput")
    gpre = nc.dram_tensor("gpre", [128, NBLK * K], BF16, kind="ExternalOutput")

    NCH = S // CH  # XT fetch chunks (one chunk covers 4 blocks)

    with tile.TileContext(nc) as tc, ExitStack() as ctx:
        const_pool = ctx.enter_context(tc.tile_pool(name="const", bufs=1))
        dense_pool = ctx.enter_context(tc.tile_pool(name="dense", bufs=1))
        dram_pool = ctx.enter_context(tc.tile_pool(name="bounce", bufs=1,
                                                   space="DRAM"))
        ep_pool = ctx.enter_context(tc.tile_pool(name="epall", bufs=1))
        e1t_pool = ctx.enter_context(tc.tile_pool(name="e1t", bufs=4))
        xt_pool = ctx.enter_context(tc.tile_pool(name="xtrows", bufs=1))
        pbf_pool = ctx.enter_context(tc.tile_pool(name="pbf", bufs=4))
        tmp_pool = ctx.enter_context(tc.tile_pool(name="tmp", bufs=4))
        acc_pool = ctx.enter_context(tc.tile_pool(name="acc", bufs=1))
        psum_p = ctx.enter_context(tc.tile_pool(name="pacc", bufs=3, space="PSUM"))
        psum_t = ctx.enter_context(tc.tile_pool(name="ptrans", bufs=2, space="PSUM"))

        ident = const_pool.tile([128, 128], BF16)
        make_identity(nc, ident[:])

        hsd_t = const_pool.tile([128, DCOL], I32)
        nc.sync.dma_start(hsd_t[:], hsd[:])
        tsd_t = const_pool.tile([128, DCOL], I32)
        nc.sync.dma_start(tsd_t[:], tsd[:])
        scat_t = const_pool.tile([128, DCOL], I32)
        nc.sync.dma_start(scat_t[:], scat[:])
        ub_t = const_pool.tile([128, NBLK * K], F32)
        nc.sync.dma_start(ub_t[:], ubt[:])

        # --- dense entity gathers (SWDGE indirect, f32 -> bf16 cast), then
        # scatters into padded slot order via a DRAM bounce.  Phase-grouped:
        # all gathers emit before the first scatter's sem-wait, so the Pool
        # sequencer never head-of-line-blocks later gathers ---
        QB = NBLK // DCOL
        # one bounce tile + one SBUF staging tile per dense column/quarter:
        # scatter q waits only column q's two gathers, and read-back q waits
        # only scatter q (shared tiles made every op wait on everything)
        bounces = []
        e12s = []
        for c in range(DCOL):
            bounces.append(dram_pool.tile([128 * (QB + 1), RW], BF16,
                                          name=f"bounce{c}", tag=f"bounce_{c}"))
            e12 = dense_pool.tile([128, RW], BF16, tag=f"e12_{c}")
            nc.vector.memset(e12[:], 0.0)
            nc.vector.memset(e12[:, D:D + 1], 1.0)
            nc.vector.memset(e12[:, E2OFF + D:E2OFF + D + 1], 1.0)
            e12s.append(e12)
        # interleave per column: the SWDGE ring drains FIFO, so scatter q's
        # descriptors must enter the ring right after column q's gathers --
        # otherwise they sit behind ALL gather descriptors and the first
        # read-back waits for the whole ~25us drain
        for c in range(DCOL):
            nc.gpsimd.indirect_dma_start(
                out=e12s[c][:, 0:D], out_offset=None, in_=ent[:, :],
                in_offset=IndirectOffsetOnAxis(ap=hsd_t[:, c:c + 1], axis=0))
            nc.gpsimd.indirect_dma_start(
                out=e12s[c][:, E2OFF:E2OFF + D], out_offset=None, in_=ent[:, :],
                in_offset=IndirectOffsetOnAxis(ap=tsd_t[:, c:c + 1], axis=0))
            nc.gpsimd.indirect_dma_start(
                out=bounces[c][:, :],
                out_offset=IndirectOffsetOnAxis(ap=scat_t[:, c:c + 1], axis=0),
                in_=e12s[c][:, :], in_offset=None)

        # --- padded slot-order rows back to SBUF + XT streaming, scheduled
        # around the HWDGE ring FIFOs: the rings drain in issue order, so
        # ep0/ep1 (which gate the first blocks) go at the HEAD of the scalar
        # ring, ahead of the odd XT chunks; ep2/ep3 gate late blocks that
        # are gather-drain-bound anyway, so they ride the sync ring after
        # the even chunks.  Partition-major bounce -> per-partition runs of
        # QB*RW*2 contiguous bytes (big spread descriptors) ---
        ep_all = ep_pool.tile([128, NBLK * RW], BF16)

        def emit_ep(c, eng):
            bv = bounces[c][:, :].rearrange("(p b) x -> p b x", b=QB + 1)
            eng.dma_start(
                ep_all[:, c * QB * RW:(c + 1) * QB * RW]
                .rearrange("p (b x) -> p b x", x=RW),
                bv[:, 0:QB, :])

        emit_ep(0, nc.scalar)
        emit_ep(1, nc.scalar)

        xt_tiles = []
        xt_engines = [nc.sync, nc.scalar]
        for g in range(NCH):
            xtt = xt_pool.tile([DAP, CH * NW], FP8, tag=f"xt{g}")
            xt_engines[g % 2].dma_start(xtt[:], xtc[:, CH * g:CH * (g + 1), :])
            xt_tiles.append(xtt)

        emit_ep(2, nc.sync)
        emit_ep(3, nc.sync)

        # bf16 g_pre accumulator: the tanh is saturated (g_pre ~ 1e3, sigma
        # ~15) so bf16 is ample, and the 16-bit reduce output keeps DVE in
        # its 2x mode
        gpre_t = acc_pool.tile([128, NBLK * K], BF16)

        # transpose the augmented heads rows -> e1~^T [101, 128], hoisted two
        # blocks ahead of the slot matmuls so the PE never waits on the
        # PSUM->SBUF cast of its own block's weights
        e1ts = {}

        def emit_tr(bb):
            tp = psum_t.tile([DA, 128], BF16, tag="tp")
            nc.tensor.transpose(out=tp[:], in_=ep_all[:, bb * RW:bb * RW + DA],
                                identity=ident[:])
            e1t = e1t_pool.tile([DA, 128], FP8, tag="e1t")
            nc.scalar.copy(e1t[:], tp[:])
            e1ts[bb] = e1t

        emit_tr(0)
        if NBLK > 1:
            emit_tr(1)

        # process blocks in PAIRS sharing one bank-aligned 2-bank PSUM tile
        # (block A at f32 cols 0:408, block B at 512:920, so each block's
        # matmul output stays inside one bank) -- one ScalarE copy and one
        # DVE multiply+reduce per pair halves the per-block fixed costs.
        # 3 pair buffers keep 6 blocks of matmuls in flight (2 starved PE)
        PW = 512  # f32 columns per pair half (2KB = one PSUM bank)
        epv2 = ep_all[:].rearrange("p (b x) -> p b x", x=RW)
        for b0 in range(0, NBLK, 2):
            pacc = psum_p.tile([128, 2 * PW], F32)
            for b in (b0, b0 + 1):
                if b + 2 < NBLK:
                    emit_tr(b + 2)
                e1t = e1ts.pop(b)
                poff = (b - b0) * PW
                xtt = xt_tiles[b // 4]
                xoff = (b % 4) * 4 * NW
                for j in range(4):
                    nc.tensor.matmul(
                        out=pacc[SLOT * j:SLOT * (j + 1), poff:poff + NW],
                        lhsT=e1t[:, SLOT * j:SLOT * (j + 1)],
                        rhs=xtt[0:DA, xoff + j * NW:xoff + (j + 1) * NW],
                        start=True, stop=True,
                        tile_position=(0, SLOT * j),
                    )

            # P to bf16 on ScalarE (closer to PSUM), then the segmented
            # multiply+reduce runs on DVE in 2x bf16 mode
            pbf = pbf_pool.tile([128, 2 * NW], BF16)
            nc.scalar.copy(
                pbf[:].rearrange("p (t x) -> p t x", t=2),
                pacc[:].rearrange("p (t w) -> p t w", t=2)[:, :, 0:NW])
            tmp = tmp_pool.tile([128, 2 * NW], BF16)
            nc.vector.tensor_tensor(
                out=tmp[:].rearrange("p (t k j) -> p t k j", t=2, k=K),
                in0=pbf[:].rearrange("p (t k j) -> p t k j", t=2, k=K),
                in1=epv2[:, b0:b0 + 2, E2OFF:E2OFF + DJ]
                    .unsqueeze(2).broadcast_to([128, 2, K, DJ]),
                op=mybir.AluOpType.mult,
            )
            with nc.allow_low_precision(reason="tanh-saturated g_pre"):
                nc.vector.reduce_sum(
                    out=gpre_t[:, K * b0:K * (b0 + 2)],
                    in_=tmp[:].rearrange("p (t k j) -> p t k j", t=2, k=K),
                    axis=mybir.AxisListType.X,
                )

        # --- batched tail: tanh, u-weighting, k-reduce, sigmoid ---
        th = const_pool.tile([128, NBLK * K], F32)
        nc.scalar.activation(th[:], gpre_t[:],
                             mybir.ActivationFunctionType.Tanh)
        scr = const_pool.tile([128, NBLK * K], F32)
        nc.vector.tensor_tensor(out=scr[:], in0=th[:], in1=ub_t[:],
                                op=mybir.AluOpType.mult)
        sco = const_pool.tile([128, NBLK], F32)
        nc.vector.reduce_sum(
            out=sco[:], in_=scr[:].rearrange("p (b k) -> p b k", k=K),
            axis=mybir.AxisListType.X)
        pred_sb = const_pool.tile([128, NBLK], F32)
        nc.scalar.activation(pred_sb[:], sco[:],
                             mybir.ActivationFunctionType.Sigmoid)
        nc.sync.dma_start(pred_t[:], pred_sb[:])
        nc.sync.dma_start(gpre[:], gpre_t[:])

    return nc


_PROGRAM_CACHE = {}


def _get_program(S, NBLK):
    key = (S, NBLK)
    if key not in _PROGRAM_CACHE:
        _PROGRAM_CACHE[key] = _build_program(S, NBLK)
    return _PROGRAM_CACHE[key]


# ---------------------------------------------------------------------------
# Entry point
# ---------------------------------------------------------------------------
def _run(inputs, trace=False, tmpdir=None, trace_cores=None):
    from concourse.bass_utils import run_bass_kernel_spmd

    heads = np.asarray(inputs["heads"]).astype(np.int64)
    tails = np.asarray(inputs["tails"]).astype(np.int64)
    relations = np.asarray(inputs["relations"]).astype(np.int64)
    ent = np.ascontiguousarray(np.asarray(inputs["entity_embedding"], np.float32))
    W = np.asarray(inputs["W"], np.float32)
    V = np.asarray(inputs["V"], np.float32)
    Bp = np.asarray(inputs["Bp"], np.float32)
    U = np.asarray(inputs["U"], np.float32)

    XTb = _build_xt(W, V, Bp)
    routed, S, NBLK = _route(heads, tails, relations, U)

    nc = _get_program(S, NBLK)

    in_maps = []
    for c in range(NCORES):
        r = routed[c]
        xtg = XTb[r["slot_rels"]]                   # [S, DA, NW] bf16
        xtg[~r["have_slot"]] = 0
        in_maps.append({
            "xtc": np.ascontiguousarray(xtg.transpose(1, 0, 2)),
            "ent": ent,
            "hsd": r["hsd"],
            "tsd": r["tsd"],
            "scat": r["scat"],
            "ubt": r["ub"],
        })

    kwargs = {}
    if trace:
        kwargs.update(trace=True, tmpdir=tmpdir)
        if trace_cores is not None:
            kwargs.update(trace_cores=trace_cores)
    res = run_bass_kernel_spmd(nc, in_maps, core_ids=list(range(NCORES)), **kwargs)

    pred = np.zeros(B, np.float32)
    for c in range(NCORES):
        pt = res.results[c]["pred_t"]  # [128, NBLK]
        for oi, b, p in routed[c]["placement"]:
            pred[oi] = pt[p, b]
    return pred, routed, res


def kernel(**inputs):
    pred, _, _ = _run(inputs)
    return pred
